# revision 7
# baseline (speedup 1.0000x reference)
"""MultiHeadAttention TRN2 Bass kernel v2, sharded over 8 NeuronCores.

Sharding: 8 cores = 2 batches x 4 head-groups (4 heads each); host sums the
per-group partial output projections.

v2 over baseline: the whole kernel is one continuous pipeline with no dead
phases, and the PE array is packed via tile_position row-pairing:
  - scores for the head PAIR run as two concurrent K=64 matmuls in the two
    row-strips of the array (tile auto-derived from partition slices), into
    one [128, 2*SUP] PSUM pair tile -> ONE exp ACTIVATE covers both heads
  - SUP=512 q-columns per instance keeps PSUM at exactly 8 banks:
    psc 2x[128,1024] (4) + out2 2x[65,512] (2) + fill 2x[128,512] (2)
  - prefix: PE warmup junk matmuls (HAM) + dummy exp (ACT table preload)
    while the first column-blocks of xk/xq/xv stream in; k/q/v projections
    start as soon as their columns land
  - all remaining projection work (k tb2-3, v tk8-15, q tb1-3) and the
    output projection drain through a filler queue inside the attention
    chunk loops, keeping PE dense while ScalarE runs the exp stream
  - DMA granularity [128,1024] (256KB) and emission order chosen so the
    ~650ns/descriptor Sync issue rate tracks the consumption schedule
  - epilogue (normalize by softmax sum): reciprocal on the [1,512] sum row
    directly, one DRAM round-trip to broadcast across partitions
"""

import numpy as np
import ml_dtypes

import concourse.bass as bass
import concourse.mybir as mybir
import concourse.tile as tile
from concourse.bacc import Bacc

BF16 = mybir.dt.bfloat16
F32 = mybir.dt.float32
nbf16 = ml_dtypes.bfloat16

B = 2
S = 2048
D = 1024
H = 16
DH = 64
HPC = 4  # heads per core
CD = HPC * DH  # 256 per-core projected dims
NCORES = 8
SCALE = 8.0  # sqrt(DH)

KC = D // 128  # 8 contraction chunks for projections
TB = 512  # projection token block
NTB = S // TB  # 4
TC = S // 128  # 16 s_k chunks
SUP = 512  # q-superblock per attention instance
NSUP = S // SUP  # 4


def build_module(debug=False):
    nc = Bacc(None)

    xqT = nc.dram_tensor("xqT", [D, S], BF16, kind="ExternalInput")
    xkT = nc.dram_tensor("xkT", [D, S], BF16, kind="ExternalInput")
    xvT = nc.dram_tensor("xvT", [D, S], BF16, kind="ExternalInput")
    wqT = nc.dram_tensor("wqT", [128, KC * CD], BF16, kind="ExternalInput")
    wkT = nc.dram_tensor("wkT", [128, KC * CD], BF16, kind="ExternalInput")
    wvT = nc.dram_tensor("wvT", [128, KC * CD], BF16, kind="ExternalInput")
    woT = nc.dram_tensor("woT", [128, (CD // 128) * D], BF16, kind="ExternalInput")
    bqc = nc.dram_tensor("bqc", [128, 2], F32, kind="ExternalInput")
    bkc = nc.dram_tensor("bkc", [128, 2], F32, kind="ExternalInput")
    expbT = nc.dram_tensor("expbT", [S, S], BF16, kind="ExternalInput")
    poutT = nc.dram_tensor("poutT", [D, S], BF16, kind="ExternalOutput")

    with tile.TileContext(nc) as tc:
        with (
            tc.tile_pool(name="statics", bufs=1) as statics,
            tc.tile_pool(name="xk", bufs=16) as xk_pool,
            tc.tile_pool(name="xq", bufs=16) as xq_pool,
            tc.tile_pool(name="expb", bufs=2) as expb_pool,
            tc.tile_pool(name="e", bufs=3) as e_pool,
            tc.tile_pool(name="a", bufs=3) as a_pool,
            tc.tile_pool(name="o2", bufs=2) as o2_pool,
            tc.tile_pool(name="rb", bufs=2) as rb_pool,
            tc.tile_pool(name="spr", bufs=2) as spr_pool,
            tc.tile_pool(name="segt", bufs=2) as seg_pool,
            tc.tile_pool(name="oev", bufs=3) as oev_pool,
            tc.tile_pool(name="psc", bufs=2, space="PSUM") as psc,
            tc.tile_pool(name="pav", bufs=2, space="PSUM") as pav,
            tc.tile_pool(name="pfill", bufs=2, space="PSUM") as pfill,
            tc.tile_pool(name="dsc", bufs=4, space="DRAM") as dram_pool,
        ):
            # ---- static tiles ----
            wq_sb = statics.tile([128, KC, CD], BF16, name="wq_sb")
            wk_sb = statics.tile([128, KC, CD], BF16, name="wk_sb")
            wv_sb = statics.tile([128, KC, CD], BF16, name="wv_sb")
            wo_sb = statics.tile([128, CD // 128, D], BF16, name="wo_sb")
            bq_sb = statics.tile([128, 2], F32, name="bq_sb")
            bk_sb = statics.tile([128, 2], F32, name="bk_sb")
            qT = [statics.tile([128, S], BF16, name=f"qT{m}") for m in range(2)]
            kpair = [statics.tile([128, S], BF16, name=f"kpair{m}") for m in range(2)]
            vv = statics.tile([128, HPC, TC, DH + 1], BF16, name="vv")
            cc = [statics.tile([128, S], BF16, name=f"cc{m}") for m in range(2)]
            xv_sb = [statics.tile([128, S], BF16, name=f"xv{k}") for k in range(KC)]
            # junk tiles for PE warmup / ACT table preload
            wjl = statics.tile([128, 128], BF16, name="wjl")
            wjr = statics.tile([128, TB], BF16, name="wjr")
            ejunk = statics.tile([128, 64], BF16, name="ejunk")

            # x tiles: [128,512] quarters for the start-critical tb0, then
            # [128,1024] halves (256KB DMAs sustain higher aggregate BW)
            xkt2 = {}  # (tb, kc) -> AP
            xqt2 = {}

            def load_x_quarter(dst, xdram, pool, tb, nm):
                for kc in range(KC):
                    t = pool.tile([128, TB], BF16, name=nm, bufs=16)
                    nc.sync.dma_start(
                        t, xdram[kc * 128 : (kc + 1) * 128, tb * TB : (tb + 1) * TB]
                    )
                    dst[(tb, kc)] = t

            def load_x_half(dst, xdram, pool, h, nm):
                for kc in range(KC):
                    t = pool.tile([128, 2 * TB], BF16, name=nm, bufs=8)
                    nc.sync.dma_start(
                        t, xdram[kc * 128 : (kc + 1) * 128, h * 1024 : (h + 1) * 1024]
                    )
                    dst[(2 * h, kc)] = t[:, 0:TB]
                    dst[(2 * h + 1, kc)] = t[:, TB : 2 * TB]

            expb_tiles = {}
            expb_srcs = [
                expbT[:, sup * SUP : (sup + 1) * SUP].rearrange(
                    "(c p) q -> p c q", p=128
                )
                for sup in range(NSUP)
            ]

            def alloc_expb(sup):
                expb_tiles[sup] = expb_pool.tile([128, TC, SUP], BF16, name="eb")

            def load_expb(sup, c0, c1):  # chunks [c0, c1)
                nc.sync.dma_start(
                    expb_tiles[sup][:, c0:c1, :], expb_srcs[sup][:, c0:c1, :]
                )

            def load_xv_col(q0, q1):  # [128, (q1-q0)*512] col-block per kc
                for kc in range(KC):
                    nc.sync.dma_start(
                        xv_sb[kc][:, q0 * TB : q1 * TB],
                        xvT[kc * 128 : (kc + 1) * 128, q0 * TB : q1 * TB],
                    )

            # ---- front DMA stream (Sync is in-order; position ~= issue time)
            nc.sync.dma_start(wk_sb, wkT[:, :].rearrange("p (kc m) -> p kc m", kc=KC))
            nc.sync.dma_start(bk_sb, bkc[:, :])
            nc.sync.dma_start(bq_sb, bqc[:, :])
            load_x_quarter(xkt2, xkT, xk_pool, 0, "xkq")  # k-proj tb0
            alloc_expb(0)
            load_expb(0, 0, 2)
            nc.sync.dma_start(wq_sb, wqT[:, :].rearrange("p (kc m) -> p kc m", kc=KC))
            load_x_quarter(xqt2, xqT, xq_pool, 0, "xqq")  # q-proj tb0 (sup0)
            load_x_quarter(xkt2, xkT, xk_pool, 1, "xkq")  # k-proj tb1
            nc.sync.dma_start(wv_sb, wvT[:, :].rearrange("p (kc m) -> p kc m", kc=KC))
            load_xv_col(0, 2)  # v-proj tk0-7
            load_expb(0, 2, 4)
            load_expb(0, 4, 8)
            load_x_half(xkt2, xkT, xk_pool, 1, "xkh")  # k-proj tb2/tb3
            load_expb(0, 8, 12)
            load_xv_col(2, 4)  # v-proj tk8-15
            load_expb(0, 12, 16)
            nc.sync.dma_start(
                wo_sb, woT[:, :].rearrange("p (kc m) -> p kc m", kc=CD // 128)
            )
            alloc_expb(1)
            for c in range(0, TC, 4):
                load_expb(1, c, c + 4)
            # q-proj tb1/tb2 in one [128,1024] block, tb3 separate
            for kc in range(KC):
                t = xq_pool.tile([128, 2 * TB], BF16, name="xqm", bufs=8)
                nc.sync.dma_start(
                    t, xqT[kc * 128 : (kc + 1) * 128, TB : 3 * TB]
                )
                xqt2[(1, kc)] = t[:, 0:TB]
                xqt2[(2, kc)] = t[:, TB : 2 * TB]
            load_x_quarter(xqt2, xqT, xq_pool, 3, "xqq")  # q-proj tb3

            # ---- PE warmup (HAM) + ACT table preload ----
            nc.gpsimd.memset(wjl[:, :], 0.0)
            nc.gpsimd.memset(wjr[:, :], 0.0)
            nc.gpsimd.memset(vv[:, :, :, DH : DH + 1], 1.0)
            for i in range(12):
                ps = pfill.tile([128, TB], F32, name="ps_junk", tag="pfill", bufs=1)
                nc.tensor.matmul(ps, lhsT=wjl, rhs=wjr, start=True, stop=True)
            nc.scalar.activation(
                ejunk, wjr[:, 0:64], func=mybir.ActivationFunctionType.Exp
            )

            # ---- projection helpers ----
            kq_open = {}

            def kq_part(xt, w_sb, b_sb, dstt, tb, mt, part):
                # half-chain quantum: 4 of 8 accumulating MMs; the open PSUM
                # tile lives in its own single-buf tag so other fill quanta
                # cannot recycle it mid-accumulation
                key = (id(dstt), tb, mt)
                if part == 0:
                    kq_open[key] = pfill.tile(
                        [128, TB], F32, name="ps_kq", tag="pkq", bufs=1
                    )
                ps = kq_open[key]
                for kc in range(part * 4, part * 4 + 4):
                    nc.tensor.matmul(
                        ps,
                        lhsT=w_sb[:, kc, mt * 128 : (mt + 1) * 128],
                        rhs=xt[(tb, kc)],
                        start=(kc == 0),
                        stop=(kc == KC - 1),
                    )
                if part == 1:
                    nc.vector.tensor_scalar_add(
                        dstt[mt][:, tb * TB : (tb + 1) * TB],
                        ps,
                        scalar1=b_sb[:, mt : mt + 1],
                    )
                    del kq_open[key]

            def kq_chain(xt, w_sb, b_sb, dstt, tb, mt):
                kq_part(xt, w_sb, b_sb, dstt, tb, mt, 0)
                kq_part(xt, w_sb, b_sb, dstt, tb, mt, 1)

            def vproj_tile(tk):
                ps = pfill.tile([128, CD], F32, name="ps_v", tag="pfill", bufs=1)
                for kc in range(KC):
                    nc.tensor.matmul(
                        ps,
                        lhsT=xv_sb[kc][:, tk * 128 : (tk + 1) * 128],
                        rhs=wv_sb[:, kc, :],
                        start=(kc == 0),
                        stop=(kc == KC - 1),
                    )
                dst = vv[:, :, tk, 0:DH]
                src = ps.rearrange("p (h d) -> p h d", h=HPC)
                # ScalarE only while it is idle (prefix); DVE once exp stream runs
                if tk < 8 and tk % 2 == 0:
                    nc.scalar.copy(dst, src)
                else:
                    nc.vector.tensor_copy(dst, src)

            op_ctr = [0]

            def outproj_tile(mo, sup, alt=False):
                # alt: alternate fill tags (pkq is free once projections end)
                # so consecutive tiles double-buffer across the two banks
                if alt and op_ctr[0] % 2 == 1:
                    ps = pfill.tile([128, SUP], F32, name="ps_o2", tag="pkq", bufs=1)
                else:
                    ps = pfill.tile([128, SUP], F32, name="ps_o", tag="pfill", bufs=1)
                op_ctr[0] += 1
                for kc in range(CD // 128):
                    nc.tensor.matmul(
                        ps,
                        lhsT=wo_sb[:, kc, mo * 128 : (mo + 1) * 128],
                        rhs=cc[kc][:, sup * SUP : (sup + 1) * SUP],
                        start=(kc == 0),
                        stop=(kc == CD // 128 - 1),
                    )
                ot = oev_pool.tile([128, SUP], BF16, name="ot")
                nc.vector.tensor_copy(ot, ps)
                nc.sync.dma_start(
                    poutT[mo * 128 : (mo + 1) * 128, sup * SUP : (sup + 1) * SUP], ot
                )

            # ---- prefix projections (in column-arrival order) ----
            for mt in range(2):
                kq_chain(xkt2, wk_sb, bk_sb, kpair, 0, mt)
            for mt in range(2):
                kq_chain(xqt2, wq_sb, bq_sb, qT, 0, mt)
            for mt in range(2):
                kq_chain(xkt2, wk_sb, bk_sb, kpair, 1, mt)
            for tk in range(8):
                vproj_tile(tk)

            # ---- filler queue: drained 1 quantum per attention chunk ----
            fillers = []

            def kq2(xt, w, b, dstt, tb, mt):
                return [
                    lambda: kq_part(xt, w, b, dstt, tb, mt, 0),
                    lambda: kq_part(xt, w, b, dstt, tb, mt, 1),
                ]

            k2m0 = kq2(xkt2, wk_sb, bk_sb, kpair, 2, 0)
            k2m1 = kq2(xkt2, wk_sb, bk_sb, kpair, 2, 1)
            k3m0 = kq2(xkt2, wk_sb, bk_sb, kpair, 3, 0)
            k3m1 = kq2(xkt2, wk_sb, bk_sb, kpair, 3, 1)
            q1m0 = kq2(xqt2, wq_sb, bq_sb, qT, 1, 0)
            q1m1 = kq2(xqt2, wq_sb, bq_sb, qT, 1, 1)
            fillers += [k2m0[0], lambda: vproj_tile(8), k2m0[1],
                        lambda: vproj_tile(9), k3m0[0], lambda: vproj_tile(10),
                        k3m0[1], lambda: vproj_tile(11), lambda: vproj_tile(12),
                        lambda: vproj_tile(13), lambda: vproj_tile(14),
                        lambda: vproj_tile(15), k2m1[0], k2m1[1],
                        k3m1[0], k3m1[1], q1m0[0], q1m0[1], q1m1[0], q1m1[1]]

            fill_pos = [0]

            def pump_filler():
                if fill_pos[0] < len(fillers):
                    fillers[fill_pos[0]]()
                    fill_pos[0] += 1

            # ---- epilogue builder (A/B combined: one DMA chain per pair)
            def make_epi(sup, mt, out2A, out2B):
                qsl = slice(sup * SUP, (sup + 1) * SUP)
                st = {}
                stages = []

                def ev(which, out2):
                    if "o2" not in st:
                        st["o2"] = o2_pool.tile(
                            [DH + 1, 2 * SUP], F32, name="o2", tag="o2", bufs=1
                        )
                    half = 0 if which == "A" else 1
                    nc.vector.tensor_copy(
                        st["o2"][:, half * SUP : (half + 1) * SUP], out2
                    )

                stages.append(lambda: ev("A", out2A))
                stages.append(lambda: ev("B", out2B))

                def dspill():
                    dr = dram_pool.tile([1, 2 * SUP], F32, name="dr", tag="dr")
                    nc.sync.dma_start(dr, st["o2"][DH : DH + 1, :])
                    st["dr"] = dr

                stages.append(dspill)

                def dspread():
                    sp = spr_pool.tile([128, 2 * SUP // 128], F32, name="sp", tag="sp")
                    nc.sync.dma_start(
                        sp, st["dr"][:, :].rearrange("a (p f) -> (a p) f", p=128)
                    )
                    st["sp"] = sp

                stages.append(dspread)

                def recip():
                    sp = st["sp"]
                    nc.vector.reciprocal(sp, sp)
                    dr2 = dram_pool.tile([1, 2 * SUP], F32, name="dr2", tag="dr2")
                    nc.sync.dma_start(
                        dr2[:, :].rearrange("a (p f) -> (a p) f", p=128), sp
                    )
                    st["dr2"] = dr2

                stages.append(recip)

                def dbcast():
                    rb = rb_pool.tile([64, 2 * SUP], F32, name="rb", tag="rb", bufs=1)
                    nc.sync.dma_start(rb, st["dr2"][:, :].partition_broadcast(64))
                    st["rb"] = rb

                stages.append(dbcast)

                def finA():  # even head -> cc partitions 0-63 directly
                    seg = cc[mt][0:64, qsl]
                    nc.vector.tensor_mul(
                        seg, st["o2"][0:DH, 0:SUP], st["rb"][:, 0:SUP]
                    )

                stages.append(finA)

                def finB():  # odd head -> partitions 64-127 via DMA move
                    segt = seg_pool.tile([64, SUP], BF16, name="segt")
                    nc.vector.tensor_mul(
                        segt, st["o2"][0:DH, SUP : 2 * SUP], st["rb"][:, SUP : 2 * SUP]
                    )
                    nc.sync.dma_start(cc[mt][64:128, qsl], segt)

                stages.append(finB)
                return stages

            EPI_AT = {0: 0, 1: 1, 2: 2, 3: 3, 5: 4, 7: 5, 9: 6, 11: 7}

            # ---- attention instances ----
            pend = None
            carry = [{}]
            insts = [(sup, mt) for sup in range(NSUP) for mt in range(2)]
            for inst, (sup, mt) in enumerate(insts):
                qsl = slice(sup * SUP, (sup + 1) * SUP)
                hA, hB = 2 * mt, 2 * mt + 1

                # stream later expb superblocks while attention runs
                if inst == 2:
                    alloc_expb(2)
                    for c in range(0, TC, 4):
                        load_expb(2, c, c + 4)
                if inst == 2:
                    for mtq in range(2):
                        fillers.extend(kq2(xqt2, wq_sb, bq_sb, qT, 2, mtq))
                if inst == 3:
                    for mtq in range(2):
                        fillers.extend(kq2(xqt2, wq_sb, bq_sb, qT, 3, mtq))
                if inst == 4:
                    alloc_expb(3)
                    for c in range(0, TC, 4):
                        load_expb(3, c, c + 4)
                # output projection for finished superblocks (appended one
                # instance after the epilogue that completes cc, so the PE
                # queue never blocks on an epilogue still in flight)
                if inst in (4, 6):
                    s_done = {4: 0, 6: 1}[inst]
                    for mo in range(D // 128):
                        fillers.append(lambda mo=mo, s=s_done: outproj_tile(mo, s))

                def emit_sc(sup_, mt_, ck, t):
                    ksl = slice(ck * 128, (ck + 1) * 128)
                    qsl_ = slice(sup_ * SUP, (sup_ + 1) * SUP)
                    nc.tensor.matmul(
                        t[:, 0:SUP],
                        lhsT=kpair[mt_][0:64, ksl],
                        rhs=qT[mt_][0:64, qsl_],
                        start=True,
                        stop=True,
                    )
                    nc.tensor.matmul(
                        t[:, SUP : 2 * SUP],
                        lhsT=kpair[mt_][64:128, ksl],
                        rhs=qT[mt_][64:128, qsl_],
                        start=True,
                        stop=True,
                    )

                if inst == 0:
                    scs = {}
                    for ck in (0, 1):
                        scs[ck] = psc.tile([128, 2 * SUP], F32, name="sc", tag="psc")
                        emit_sc(sup, mt, ck, scs[ck])
                else:
                    scs = carry[0]
                    carry[0] = {}
                out2A = pav.tile([DH + 1, SUP], F32, name="o2A", tag="pav")
                out2B = pav.tile([DH + 1, SUP], F32, name="o2B", tag="pav")

                for ck in range(TC):
                    sc = scs.pop(ck)
                    e = e_pool.tile([128, 2 * SUP], BF16, name="e")
                    nc.scalar.activation(
                        e, sc, func=mybir.ActivationFunctionType.Exp
                    )
                    if pend is not None and ck in EPI_AT:
                        pend[EPI_AT[ck]]()
                    # one fused mul: expb chunk repeats across both heads
                    ebr = expb_tiles[sup][:, ck, None, :].broadcast_to([128, 2, SUP])
                    a = a_pool.tile([128, 2 * SUP], BF16, name="a")
                    nc.vector.tensor_mul(
                        a.rearrange("p (o f) -> p o f", o=2),
                        e.rearrange("p (o f) -> p o f", o=2),
                        ebr,
                    )
                    if ck + 2 < TC:
                        scs[ck + 2] = psc.tile(
                            [128, 2 * SUP], F32, name="sc", tag="psc"
                        )
                        emit_sc(sup, mt, ck + 2, scs[ck + 2])
                    elif inst + 1 < len(insts):
                        # prefetch next instance's first scores so the exp
                        # stream never waits on the instance boundary
                        nsup, nmt = insts[inst + 1]
                        nk = ck + 2 - TC
                        t = psc.tile([128, 2 * SUP], F32, name="sc", tag="psc")
                        emit_sc(nsup, nmt, nk, t)
                        carry[0][nk] = t
                    nc.tensor.matmul(
                        out2A,
                        lhsT=vv[:, hA, ck, :],
                        rhs=a[:, 0:SUP],
                        start=(ck == 0),
                        stop=(ck == TC - 1),
                    )
                    nc.tensor.matmul(
                        out2B,
                        lhsT=vv[:, hB, ck, :],
                        rhs=a[:, SUP : 2 * SUP],
                        start=(ck == 0),
                        stop=(ck == TC - 1),
                    )
                    if inst > 0 or ck >= 2:
                        pump_filler()

                pend = make_epi(sup, mt, out2A, out2B)

            # ---- tail: last epilogue chain (DVE/DMA) runs while PE does
            # sup2's output projection; sup3 follows once cc is complete
            while fill_pos[0] < len(fillers):
                pump_filler()
            # sup2 outproj first: emitted before the s3 epilogue's cc writes
            # so it carries no dependency on the (serial) DMA chain and keeps
            # PE busy underneath it
            for stage in pend[:2]:
                stage()
            for mo in range(D // 128):
                outproj_tile(mo, NSUP - 2, alt=True)
            for stage in pend[2:]:
                stage()
            pend = None
            for mo in range(D // 128):
                outproj_tile(mo, NSUP - 1, alt=True)

    nc.finalize()
    return nc


def make_in_maps(query, key, value, mask, chemical_bias, Wq, bq, Wk, bk, Wv, bv, Wo):
    """Host-side preprocessing: per-core input dicts (8 cores)."""
    f32 = np.float32

    def c(a, dt):
        return np.ascontiguousarray(a, dtype=dt)

    per_batch = []
    for b in range(B):
        xq = c(query[b].T, nbf16)
        xk = c(key[b].T, nbf16)
        xv = c(value[b].T, nbf16)
        bm = np.where(mask[b, 0] == 0, f32(0.0), np.exp(chemical_bias[b], dtype=f32))
        expbT_ = c(bm.T, nbf16)
        per_batch.append((xq, xk, xv, expbT_))

    def warr(wt, kc):
        # [kc*128, M] -> [128, kc*M]: per-partition-contiguous device layout
        m = wt.shape[1]
        return np.ascontiguousarray(
            wt.reshape(kc, 128, m).transpose(1, 0, 2).reshape(128, kc * m), nbf16
        )

    per_group = []
    for g in range(4):
        hsl = slice(g * CD, (g + 1) * CD)
        wqT_ = warr(np.asarray((Wq[hsl] / SCALE).T, np.float32), KC)
        wkT_ = warr(np.asarray(Wk[hsl].T, np.float32), KC)
        wvT_ = warr(np.asarray(Wv[hsl].T, np.float32), KC)
        woT_ = warr(np.asarray(Wo[:, hsl].T, np.float32), CD // 128)
        bqc_ = c((bq[hsl] / SCALE).reshape(2, 128).T, f32)
        bkc_ = c(bk[hsl].reshape(2, 128).T, f32)
        per_group.append((wqT_, wkT_, wvT_, woT_, bqc_, bkc_))

    in_maps = []
    for core in range(NCORES):
        b, g = divmod(core, 4)
        xq, xk, xv, expbT_ = per_batch[b]
        wqT_, wkT_, wvT_, woT_, bqc_, bkc_ = per_group[g]
        in_maps.append(
            {
                "xqT": xq,
                "xkT": xk,
                "xvT": xv,
                "wqT": wqT_,
                "wkT": wkT_,
                "wvT": wvT_,
                "woT": woT_,
                "bqc": bqc_,
                "bkc": bkc_,
                "expbT": expbT_,
            }
        )
    return in_maps


def combine_outputs(results, bo_eff):
    """Sum per-group transposed partials into the full [B, S, D] output."""
    out = np.empty((B, S, D), np.float32)
    for b in range(B):
        acc = results[4 * b]["poutT"].T.astype(np.float32)
        for g in range(1, 4):
            acc = acc + results[4 * b + g]["poutT"].T.astype(np.float32)
        out[b] = acc + bo_eff
    return out


_NC_CACHE = {}


def _get_module(debug=False):
    key = debug
    if key not in _NC_CACHE:
        _NC_CACHE[key] = build_module(debug=debug)
    return _NC_CACHE[key]


def run_spmd(in_maps, debug=False, **kwargs):
    from concourse.bass_utils import run_bass_kernel_spmd

    nc = _get_module(debug)
    return run_bass_kernel_spmd(nc, in_maps, core_ids=list(range(NCORES)), **kwargs)


def kernel(query, key, value, mask, chemical_bias, Wq, bq, Wk, bk, Wv, bv, Wo, bo):
    in_maps = make_in_maps(
        query, key, value, mask, chemical_bias, Wq, bq, Wk, bk, Wv, bv, Wo
    )
    res = run_spmd(in_maps)
    # v-bias folded out of the device epilogue: out += Wo @ bv per column
    bo_eff = (
        bo.astype(np.float32) + Wo.astype(np.float32) @ bv.astype(np.float32)
    )
    return combine_outputs(res.results, bo_eff)


# revision 8
# speedup vs baseline: 1.1715x; 1.1715x over previous
"""MultiHeadAttention TRN2 Bass kernel, sharded over 8 NeuronCores.

Sharding: 8 cores = 2 batches x 4 head-groups (4 heads each); host sums the
per-group partial output projections (v-bias folded into bo host-side).

One continuous pipeline; the PE array is packed via tile_position pairing:
  - scores for the head PAIR run as two concurrent K=64 matmuls in the two
    row-strips of the array (tile auto-derived from partition slices), into
    one [128, 2*SUP] PSUM pair tile -> ONE exp ACTIVATE covers both heads,
    and one DVE mul (0-stride broadcast of the expb chunk) rescales both
  - SUP=512 q-columns per instance keeps PSUM at exactly 8 banks:
    psc 2x[128,1024] (4) + out2 2x[65,512] (2) + fill pkq/pfill (2)
  - prefix: PE warmup junk matmuls (HAM) + dummy exp (ACT table preload)
    while the first column-blocks of xk/xq/xv stream in; k/q/v projections
    start as soon as their columns land
  - all remaining projection work (k tb2-3, v tk8-15, q tb1-3) and the
    output projection drain through a deadline-ordered filler queue inside
    the attention chunk loops, in 4-matmul quanta so the in-order PE queue
    never blocks long; the next instance's first two score pairs are
    emitted at chunks 14/15 so exp never waits on an instance boundary
  - DMA sizes/order chosen so the ~620ns/descriptor Sync issue rate tracks
    consumption; A/B epilogues share one [65,1024] evac + one reciprocal
    DMA chain (spill -> spread -> wide reciprocal -> broadcast)
  - tail: sup2 output projection runs on PE underneath the last epilogue's
    serial DMA chain; outproj double-buffers across the two fill banks
"""

import numpy as np
import ml_dtypes

import concourse.bass as bass
import concourse.mybir as mybir
import concourse.tile as tile
from concourse.bacc import Bacc

BF16 = mybir.dt.bfloat16
F32 = mybir.dt.float32
nbf16 = ml_dtypes.bfloat16

B = 2
S = 2048
D = 1024
H = 16
DH = 64
HPC = 4  # heads per core
CD = HPC * DH  # 256 per-core projected dims
NCORES = 8
SCALE = 8.0  # sqrt(DH)

KC = D // 128  # 8 contraction chunks for projections
TB = 512  # projection token block
NTB = S // TB  # 4
TC = S // 128  # 16 s_k chunks
SUP = 512  # q-superblock per attention instance
NSUP = S // SUP  # 4


def build_module(debug=False):
    nc = Bacc(None)

    xqT = nc.dram_tensor("xqT", [D, S], BF16, kind="ExternalInput")
    xkT = nc.dram_tensor("xkT", [D, S], BF16, kind="ExternalInput")
    xvT = nc.dram_tensor("xvT", [D, S], BF16, kind="ExternalInput")
    wqT = nc.dram_tensor("wqT", [128, KC * CD], BF16, kind="ExternalInput")
    wkT = nc.dram_tensor("wkT", [128, KC * CD], BF16, kind="ExternalInput")
    wvT = nc.dram_tensor("wvT", [128, KC * CD], BF16, kind="ExternalInput")
    woT = nc.dram_tensor("woT", [128, (CD // 128) * D], BF16, kind="ExternalInput")
    bqc = nc.dram_tensor("bqc", [128, 2], F32, kind="ExternalInput")
    bkc = nc.dram_tensor("bkc", [128, 2], F32, kind="ExternalInput")
    expbT = nc.dram_tensor("expbT", [S, S], BF16, kind="ExternalInput")
    poutT = nc.dram_tensor("poutT", [D, S], BF16, kind="ExternalOutput")

    with tile.TileContext(nc) as tc:
        with (
            tc.tile_pool(name="statics", bufs=1) as statics,
            tc.tile_pool(name="xk", bufs=16) as xk_pool,
            tc.tile_pool(name="xq", bufs=16) as xq_pool,
            tc.tile_pool(name="expb", bufs=2) as expb_pool,
            tc.tile_pool(name="e", bufs=3) as e_pool,
            tc.tile_pool(name="a", bufs=3) as a_pool,
            tc.tile_pool(name="o2", bufs=2) as o2_pool,
            tc.tile_pool(name="rb", bufs=2) as rb_pool,
            tc.tile_pool(name="spr", bufs=2) as spr_pool,
            tc.tile_pool(name="segt", bufs=2) as seg_pool,
            tc.tile_pool(name="oev", bufs=3) as oev_pool,
            tc.tile_pool(name="psc", bufs=2, space="PSUM") as psc,
            tc.tile_pool(name="pav", bufs=2, space="PSUM") as pav,
            tc.tile_pool(name="pfill", bufs=2, space="PSUM") as pfill,
            tc.tile_pool(name="dsc", bufs=4, space="DRAM") as dram_pool,
        ):
            # ---- static tiles ----
            wq_sb = statics.tile([128, KC, CD], BF16, name="wq_sb")
            wk_sb = statics.tile([128, KC, CD], BF16, name="wk_sb")
            wv_sb = statics.tile([128, KC, CD], BF16, name="wv_sb")
            wo_sb = statics.tile([128, CD // 128, D], BF16, name="wo_sb")
            bq_sb = statics.tile([128, 2], F32, name="bq_sb")
            bk_sb = statics.tile([128, 2], F32, name="bk_sb")
            qT = [statics.tile([128, S], BF16, name=f"qT{m}") for m in range(2)]
            kpair = [statics.tile([128, S], BF16, name=f"kpair{m}") for m in range(2)]
            vv = statics.tile([128, HPC, TC, DH + 1], BF16, name="vv")
            cc = [statics.tile([128, S], BF16, name=f"cc{m}") for m in range(2)]
            xv_sb = [statics.tile([128, S], BF16, name=f"xv{k}") for k in range(KC)]
            # junk tiles for PE warmup / ACT table preload
            wjl = statics.tile([128, 128], BF16, name="wjl")
            wjr = statics.tile([128, TB], BF16, name="wjr")
            ejunk = statics.tile([128, 64], BF16, name="ejunk")

            # x tiles: [128,512] quarters for the start-critical tb0, then
            # [128,1024] halves (256KB DMAs sustain higher aggregate BW)
            xkt2 = {}  # (tb, kc) -> AP
            xqt2 = {}

            def load_x_quarter(dst, xdram, pool, tb, nm):
                for kc in range(KC):
                    t = pool.tile([128, TB], BF16, name=nm, bufs=16)
                    nc.sync.dma_start(
                        t, xdram[kc * 128 : (kc + 1) * 128, tb * TB : (tb + 1) * TB]
                    )
                    dst[(tb, kc)] = t

            def load_x_half(dst, xdram, pool, h, nm):
                for kc in range(KC):
                    t = pool.tile([128, 2 * TB], BF16, name=nm, bufs=8)
                    nc.sync.dma_start(
                        t, xdram[kc * 128 : (kc + 1) * 128, h * 1024 : (h + 1) * 1024]
                    )
                    dst[(2 * h, kc)] = t[:, 0:TB]
                    dst[(2 * h + 1, kc)] = t[:, TB : 2 * TB]

            expb_tiles = {}
            expb_srcs = [
                expbT[:, sup * SUP : (sup + 1) * SUP].rearrange(
                    "(c p) q -> p c q", p=128
                )
                for sup in range(NSUP)
            ]

            def alloc_expb(sup):
                expb_tiles[sup] = expb_pool.tile([128, TC, SUP], BF16, name="eb")

            def load_expb(sup, c0, c1):  # chunks [c0, c1)
                nc.sync.dma_start(
                    expb_tiles[sup][:, c0:c1, :], expb_srcs[sup][:, c0:c1, :]
                )

            def load_xv_col(q0, q1):  # [128, (q1-q0)*512] col-block per kc
                for kc in range(KC):
                    nc.sync.dma_start(
                        xv_sb[kc][:, q0 * TB : q1 * TB],
                        xvT[kc * 128 : (kc + 1) * 128, q0 * TB : q1 * TB],
                    )

            # ---- front DMA stream (Sync is in-order; position ~= issue time)
            nc.sync.dma_start(wk_sb, wkT[:, :].rearrange("p (kc m) -> p kc m", kc=KC))
            nc.sync.dma_start(bk_sb, bkc[:, :])
            nc.sync.dma_start(bq_sb, bqc[:, :])
            load_x_quarter(xkt2, xkT, xk_pool, 0, "xkq")  # k-proj tb0
            alloc_expb(0)
            load_expb(0, 0, 2)
            nc.sync.dma_start(wq_sb, wqT[:, :].rearrange("p (kc m) -> p kc m", kc=KC))
            load_x_quarter(xqt2, xqT, xq_pool, 0, "xqq")  # q-proj tb0 (sup0)
            load_x_quarter(xkt2, xkT, xk_pool, 1, "xkq")  # k-proj tb1
            nc.sync.dma_start(wv_sb, wvT[:, :].rearrange("p (kc m) -> p kc m", kc=KC))
            load_xv_col(0, 2)  # v-proj tk0-7
            load_expb(0, 2, 4)
            load_expb(0, 4, 8)
            load_x_half(xkt2, xkT, xk_pool, 1, "xkh")  # k-proj tb2/tb3
            load_expb(0, 8, 12)
            load_xv_col(2, 4)  # v-proj tk8-15
            load_expb(0, 12, 16)
            nc.sync.dma_start(
                wo_sb, woT[:, :].rearrange("p (kc m) -> p kc m", kc=CD // 128)
            )
            alloc_expb(1)
            for c in range(0, TC, 4):
                load_expb(1, c, c + 4)
            # q-proj tb1/tb2 in one [128,1024] block, tb3 separate
            for kc in range(KC):
                t = xq_pool.tile([128, 2 * TB], BF16, name="xqm", bufs=8)
                nc.sync.dma_start(
                    t, xqT[kc * 128 : (kc + 1) * 128, TB : 3 * TB]
                )
                xqt2[(1, kc)] = t[:, 0:TB]
                xqt2[(2, kc)] = t[:, TB : 2 * TB]
            load_x_quarter(xqt2, xqT, xq_pool, 3, "xqq")  # q-proj tb3

            # ---- PE warmup (HAM) + ACT table preload ----
            nc.gpsimd.memset(wjl[:, :], 0.0)
            nc.gpsimd.memset(wjr[:, :], 0.0)
            nc.gpsimd.memset(vv[:, :, :, DH : DH + 1], 1.0)
            for i in range(12):
                ps = pfill.tile([128, TB], F32, name="ps_junk", tag="pfill", bufs=1)
                nc.tensor.matmul(ps, lhsT=wjl, rhs=wjr, start=True, stop=True)
            nc.scalar.activation(
                ejunk, wjr[:, 0:64], func=mybir.ActivationFunctionType.Exp
            )

            # ---- projection helpers ----
            kq_open = {}

            def kq_part(xt, w_sb, b_sb, dstt, tb, mt, part):
                # half-chain quantum: 4 of 8 accumulating MMs; the open PSUM
                # tile lives in its own single-buf tag so other fill quanta
                # cannot recycle it mid-accumulation
                key = (id(dstt), tb, mt)
                if part == 0:
                    kq_open[key] = pfill.tile(
                        [128, TB], F32, name="ps_kq", tag="pkq", bufs=1
                    )
                ps = kq_open[key]
                for kc in range(part * 4, part * 4 + 4):
                    nc.tensor.matmul(
                        ps,
                        lhsT=w_sb[:, kc, mt * 128 : (mt + 1) * 128],
                        rhs=xt[(tb, kc)],
                        start=(kc == 0),
                        stop=(kc == KC - 1),
                    )
                if part == 1:
                    nc.vector.tensor_scalar_add(
                        dstt[mt][:, tb * TB : (tb + 1) * TB],
                        ps,
                        scalar1=b_sb[:, mt : mt + 1],
                    )
                    del kq_open[key]

            def kq_chain(xt, w_sb, b_sb, dstt, tb, mt):
                kq_part(xt, w_sb, b_sb, dstt, tb, mt, 0)
                kq_part(xt, w_sb, b_sb, dstt, tb, mt, 1)

            def vproj_tile(tk):
                ps = pfill.tile([128, CD], F32, name="ps_v", tag="pfill", bufs=1)
                for kc in range(KC):
                    nc.tensor.matmul(
                        ps,
                        lhsT=xv_sb[kc][:, tk * 128 : (tk + 1) * 128],
                        rhs=wv_sb[:, kc, :],
                        start=(kc == 0),
                        stop=(kc == KC - 1),
                    )
                dst = vv[:, :, tk, 0:DH]
                src = ps.rearrange("p (h d) -> p h d", h=HPC)
                # ScalarE only while it is idle (prefix); DVE once exp stream runs
                if tk < 8 and tk % 2 == 0:
                    nc.scalar.copy(dst, src)
                else:
                    nc.vector.tensor_copy(dst, src)

            op_ctr = [0]

            def outproj_tile(mo, sup, alt=False):
                # alt: alternate fill tags (pkq is free once projections end)
                # so consecutive tiles double-buffer across the two banks
                if alt and op_ctr[0] % 2 == 1:
                    ps = pfill.tile([128, SUP], F32, name="ps_o2", tag="pkq", bufs=1)
                else:
                    ps = pfill.tile([128, SUP], F32, name="ps_o", tag="pfill", bufs=1)
                op_ctr[0] += 1
                for kc in range(CD // 128):
                    nc.tensor.matmul(
                        ps,
                        lhsT=wo_sb[:, kc, mo * 128 : (mo + 1) * 128],
                        rhs=cc[kc][:, sup * SUP : (sup + 1) * SUP],
                        start=(kc == 0),
                        stop=(kc == CD // 128 - 1),
                    )
                ot = oev_pool.tile([128, SUP], BF16, name="ot")
                nc.vector.tensor_copy(ot, ps)
                nc.sync.dma_start(
                    poutT[mo * 128 : (mo + 1) * 128, sup * SUP : (sup + 1) * SUP], ot
                )

            # ---- prefix projections (in column-arrival order) ----
            for mt in range(2):
                kq_chain(xkt2, wk_sb, bk_sb, kpair, 0, mt)
            for mt in range(2):
                kq_chain(xqt2, wq_sb, bq_sb, qT, 0, mt)
            for mt in range(2):
                kq_chain(xkt2, wk_sb, bk_sb, kpair, 1, mt)
            for tk in range(8):
                vproj_tile(tk)

            # ---- filler queue: drained 1 quantum per attention chunk ----
            fillers = []

            def kq2(xt, w, b, dstt, tb, mt):
                return [
                    lambda: kq_part(xt, w, b, dstt, tb, mt, 0),
                    lambda: kq_part(xt, w, b, dstt, tb, mt, 1),
                ]

            k2m0 = kq2(xkt2, wk_sb, bk_sb, kpair, 2, 0)
            k2m1 = kq2(xkt2, wk_sb, bk_sb, kpair, 2, 1)
            k3m0 = kq2(xkt2, wk_sb, bk_sb, kpair, 3, 0)
            k3m1 = kq2(xkt2, wk_sb, bk_sb, kpair, 3, 1)
            q1m0 = kq2(xqt2, wq_sb, bq_sb, qT, 1, 0)
            q1m1 = kq2(xqt2, wq_sb, bq_sb, qT, 1, 1)
            fillers += [k2m0[0], lambda: vproj_tile(8), k2m0[1],
                        lambda: vproj_tile(9), k3m0[0], lambda: vproj_tile(10),
                        k3m0[1], lambda: vproj_tile(11), lambda: vproj_tile(12),
                        lambda: vproj_tile(13), lambda: vproj_tile(14),
                        lambda: vproj_tile(15), k2m1[0], k2m1[1],
                        k3m1[0], k3m1[1], q1m0[0], q1m0[1], q1m1[0], q1m1[1]]

            fill_pos = [0]

            def pump_filler():
                if fill_pos[0] < len(fillers):
                    fillers[fill_pos[0]]()
                    fill_pos[0] += 1

            # ---- epilogue builder (A/B combined: one DMA chain per pair)
            def make_epi(sup, mt, out2A, out2B):
                qsl = slice(sup * SUP, (sup + 1) * SUP)
                st = {}
                stages = []

                def ev(which, out2):
                    if "o2" not in st:
                        st["o2"] = o2_pool.tile(
                            [DH + 1, 2 * SUP], F32, name="o2", tag="o2", bufs=1
                        )
                    half = 0 if which == "A" else 1
                    nc.vector.tensor_copy(
                        st["o2"][:, half * SUP : (half + 1) * SUP], out2
                    )

                stages.append(lambda: ev("A", out2A))
                stages.append(lambda: ev("B", out2B))

                def dspill():
                    dr = dram_pool.tile([1, 2 * SUP], F32, name="dr", tag="dr")
                    nc.sync.dma_start(dr, st["o2"][DH : DH + 1, :])
                    st["dr"] = dr

                stages.append(dspill)

                def dspread():
                    sp = spr_pool.tile([128, 2 * SUP // 128], F32, name="sp", tag="sp")
                    nc.sync.dma_start(
                        sp, st["dr"][:, :].rearrange("a (p f) -> (a p) f", p=128)
                    )
                    st["sp"] = sp

                stages.append(dspread)

                def recip():
                    sp = st["sp"]
                    nc.vector.reciprocal(sp, sp)
                    dr2 = dram_pool.tile([1, 2 * SUP], F32, name="dr2", tag="dr2")
                    nc.sync.dma_start(
                        dr2[:, :].rearrange("a (p f) -> (a p) f", p=128), sp
                    )
                    st["dr2"] = dr2

                stages.append(recip)

                def dbcast():
                    rb = rb_pool.tile([64, 2 * SUP], F32, name="rb", tag="rb", bufs=1)
                    nc.sync.dma_start(rb, st["dr2"][:, :].partition_broadcast(64))
                    st["rb"] = rb

                stages.append(dbcast)

                def finA():  # even head -> cc partitions 0-63 directly
                    seg = cc[mt][0:64, qsl]
                    nc.vector.tensor_mul(
                        seg, st["o2"][0:DH, 0:SUP], st["rb"][:, 0:SUP]
                    )

                stages.append(finA)

                def finB():  # odd head -> partitions 64-127 via DMA move
                    segt = seg_pool.tile([64, SUP], BF16, name="segt")
                    nc.vector.tensor_mul(
                        segt, st["o2"][0:DH, SUP : 2 * SUP], st["rb"][:, SUP : 2 * SUP]
                    )
                    nc.sync.dma_start(cc[mt][64:128, qsl], segt)

                stages.append(finB)
                return stages

            EPI_AT = {0: 0, 1: 1, 2: 2, 3: 3, 5: 4, 7: 5, 9: 6, 11: 7}

            # ---- attention instances ----
            pend = None
            carry = [{}]
            insts = [(sup, mt) for sup in range(NSUP) for mt in range(2)]
            for inst, (sup, mt) in enumerate(insts):
                qsl = slice(sup * SUP, (sup + 1) * SUP)
                hA, hB = 2 * mt, 2 * mt + 1

                # stream later expb superblocks while attention runs
                if inst == 2:
                    alloc_expb(2)
                    for c in range(0, TC, 4):
                        load_expb(2, c, c + 4)
                if inst == 2:
                    for mtq in range(2):
                        fillers.extend(kq2(xqt2, wq_sb, bq_sb, qT, 2, mtq))
                if inst == 3:
                    for mtq in range(2):
                        fillers.extend(kq2(xqt2, wq_sb, bq_sb, qT, 3, mtq))
                if inst == 4:
                    alloc_expb(3)
                    for c in range(0, TC, 4):
                        load_expb(3, c, c + 4)
                # output projection for finished superblocks (appended one
                # instance after the epilogue that completes cc, so the PE
                # queue never blocks on an epilogue still in flight)
                if inst in (4, 6):
                    s_done = {4: 0, 6: 1}[inst]
                    for mo in range(D // 128):
                        fillers.append(lambda mo=mo, s=s_done: outproj_tile(mo, s))

                def emit_sc(sup_, mt_, ck, t):
                    ksl = slice(ck * 128, (ck + 1) * 128)
                    qsl_ = slice(sup_ * SUP, (sup_ + 1) * SUP)
                    nc.tensor.matmul(
                        t[:, 0:SUP],
                        lhsT=kpair[mt_][0:64, ksl],
                        rhs=qT[mt_][0:64, qsl_],
                        start=True,
                        stop=True,
                    )
                    nc.tensor.matmul(
                        t[:, SUP : 2 * SUP],
                        lhsT=kpair[mt_][64:128, ksl],
                        rhs=qT[mt_][64:128, qsl_],
                        start=True,
                        stop=True,
                    )

                if inst == 0:
                    scs = {}
                    for ck in (0, 1):
                        scs[ck] = psc.tile([128, 2 * SUP], F32, name="sc", tag="psc")
                        emit_sc(sup, mt, ck, scs[ck])
                else:
                    scs = carry[0]
                    carry[0] = {}
                out2A = pav.tile([DH + 1, SUP], F32, name="o2A", tag="pav")
                out2B = pav.tile([DH + 1, SUP], F32, name="o2B", tag="pav")

                for ck in range(TC):
                    sc = scs.pop(ck)
                    e = e_pool.tile([128, 2 * SUP], BF16, name="e")
                    nc.scalar.activation(
                        e, sc, func=mybir.ActivationFunctionType.Exp
                    )
                    if pend is not None and ck in EPI_AT:
                        pend[EPI_AT[ck]]()
                    # one fused mul: expb chunk repeats across both heads
                    ebr = expb_tiles[sup][:, ck, None, :].broadcast_to([128, 2, SUP])
                    a = a_pool.tile([128, 2 * SUP], BF16, name="a")
                    nc.vector.tensor_mul(
                        a.rearrange("p (o f) -> p o f", o=2),
                        e.rearrange("p (o f) -> p o f", o=2),
                        ebr,
                    )
                    if ck + 2 < TC:
                        scs[ck + 2] = psc.tile(
                            [128, 2 * SUP], F32, name="sc", tag="psc"
                        )
                        emit_sc(sup, mt, ck + 2, scs[ck + 2])
                    elif inst + 1 < len(insts):
                        # prefetch next instance's first scores so the exp
                        # stream never waits on the instance boundary
                        nsup, nmt = insts[inst + 1]
                        nk = ck + 2 - TC
                        t = psc.tile([128, 2 * SUP], F32, name="sc", tag="psc")
                        emit_sc(nsup, nmt, nk, t)
                        carry[0][nk] = t
                    nc.tensor.matmul(
                        out2A,
                        lhsT=vv[:, hA, ck, :],
                        rhs=a[:, 0:SUP],
                        start=(ck == 0),
                        stop=(ck == TC - 1),
                    )
                    nc.tensor.matmul(
                        out2B,
                        lhsT=vv[:, hB, ck, :],
                        rhs=a[:, SUP : 2 * SUP],
                        start=(ck == 0),
                        stop=(ck == TC - 1),
                    )
                    if inst > 0 or ck >= 2:
                        pump_filler()

                pend = make_epi(sup, mt, out2A, out2B)

            # ---- tail: last epilogue chain (DVE/DMA) runs while PE does
            # sup2's output projection; sup3 follows once cc is complete
            while fill_pos[0] < len(fillers):
                pump_filler()
            # sup2 outproj first: emitted before the s3 epilogue's cc writes
            # so it carries no dependency on the (serial) DMA chain and keeps
            # PE busy underneath it
            for stage in pend[:2]:
                stage()
            for mo in range(D // 128):
                outproj_tile(mo, NSUP - 2, alt=True)
            for stage in pend[2:]:
                stage()
            pend = None
            for mo in range(D // 128):
                outproj_tile(mo, NSUP - 1, alt=True)

    nc.finalize()
    return nc


def make_in_maps(query, key, value, mask, chemical_bias, Wq, bq, Wk, bk, Wv, bv, Wo):
    """Host-side preprocessing: per-core input dicts (8 cores)."""
    f32 = np.float32

    def c(a, dt):
        return np.ascontiguousarray(a, dtype=dt)

    per_batch = []
    for b in range(B):
        xq = c(query[b].T, nbf16)
        xk = c(key[b].T, nbf16)
        xv = c(value[b].T, nbf16)
        bm = np.where(mask[b, 0] == 0, f32(0.0), np.exp(chemical_bias[b], dtype=f32))
        expbT_ = c(bm.T, nbf16)
        per_batch.append((xq, xk, xv, expbT_))

    def warr(wt, kc):
        # [kc*128, M] -> [128, kc*M]: per-partition-contiguous device layout
        m = wt.shape[1]
        return np.ascontiguousarray(
            wt.reshape(kc, 128, m).transpose(1, 0, 2).reshape(128, kc * m), nbf16
        )

    per_group = []
    for g in range(4):
        hsl = slice(g * CD, (g + 1) * CD)
        wqT_ = warr(np.asarray((Wq[hsl] / SCALE).T, np.float32), KC)
        wkT_ = warr(np.asarray(Wk[hsl].T, np.float32), KC)
        wvT_ = warr(np.asarray(Wv[hsl].T, np.float32), KC)
        woT_ = warr(np.asarray(Wo[:, hsl].T, np.float32), CD // 128)
        bqc_ = c((bq[hsl] / SCALE).reshape(2, 128).T, f32)
        bkc_ = c(bk[hsl].reshape(2, 128).T, f32)
        per_group.append((wqT_, wkT_, wvT_, woT_, bqc_, bkc_))

    in_maps = []
    for core in range(NCORES):
        b, g = divmod(core, 4)
        xq, xk, xv, expbT_ = per_batch[b]
        wqT_, wkT_, wvT_, woT_, bqc_, bkc_ = per_group[g]
        in_maps.append(
            {
                "xqT": xq,
                "xkT": xk,
                "xvT": xv,
                "wqT": wqT_,
                "wkT": wkT_,
                "wvT": wvT_,
                "woT": woT_,
                "bqc": bqc_,
                "bkc": bkc_,
                "expbT": expbT_,
            }
        )
    return in_maps


def combine_outputs(results, bo_eff):
    """Sum per-group transposed partials into the full [B, S, D] output."""
    out = np.empty((B, S, D), np.float32)
    for b in range(B):
        acc = results[4 * b]["poutT"].T.astype(np.float32)
        for g in range(1, 4):
            acc = acc + results[4 * b + g]["poutT"].T.astype(np.float32)
        out[b] = acc + bo_eff
    return out


_NC_CACHE = {}


def _get_module(debug=False):
    key = debug
    if key not in _NC_CACHE:
        _NC_CACHE[key] = build_module(debug=debug)
    return _NC_CACHE[key]


def run_spmd(in_maps, debug=False, **kwargs):
    from concourse.bass_utils import run_bass_kernel_spmd

    nc = _get_module(debug)
    return run_bass_kernel_spmd(nc, in_maps, core_ids=list(range(NCORES)), **kwargs)


def kernel(query, key, value, mask, chemical_bias, Wq, bq, Wk, bk, Wv, bv, Wo, bo):
    in_maps = make_in_maps(
        query, key, value, mask, chemical_bias, Wq, bq, Wk, bk, Wv, bv, Wo
    )
    res = run_spmd(in_maps)
    # v-bias folded out of the device epilogue: out += Wo @ bv per column
    bo_eff = (
        bo.astype(np.float32) + Wo.astype(np.float32) @ bv.astype(np.float32)
    )
    return combine_outputs(res.results, bo_eff)


# revision 9
# speedup vs baseline: 1.1779x; 1.0055x over previous
"""MultiHeadAttention TRN2 Bass kernel, sharded over 8 NeuronCores.

Sharding: 8 cores = 2 batches x 4 head-groups (4 heads each); host sums the
per-group partial output projections (v-bias folded into bo host-side).

One continuous pipeline; the PE array is packed via tile_position pairing:
  - scores for the head PAIR run as two concurrent K=64 matmuls in the two
    row-strips of the array (tile auto-derived from partition slices), into
    one [128, 2*SUP] PSUM pair tile -> ONE exp ACTIVATE covers both heads,
    and one DVE mul (0-stride broadcast of the expb chunk) rescales both
  - SUP=512 q-columns per instance keeps PSUM at exactly 8 banks:
    psc 2x[128,1024] (4) + out2 2x[65,512] (2) + fill pkq/pfill (2)
  - prefix: PE warmup junk matmuls (HAM) + dummy exp (ACT table preload)
    while the first column-blocks of xk/xq/xv stream in; k/q/v projections
    start as soon as their columns land
  - all remaining projection work (k tb2-3, v tk8-15, q tb1-3) and the
    output projection drain through a deadline-ordered filler queue inside
    the attention chunk loops, in 4-matmul quanta so the in-order PE queue
    never blocks long; the next instance's first two score pairs are
    emitted at chunks 14/15 so exp never waits on an instance boundary
  - DMA sizes/order chosen so the ~620ns/descriptor Sync issue rate tracks
    consumption; A/B epilogues share one [65,1024] evac + one reciprocal
    DMA chain (spill -> spread -> wide reciprocal -> broadcast)
  - tail: sup2 output projection runs on PE underneath the last epilogue's
    serial DMA chain; outproj double-buffers across the two fill banks
"""

import numpy as np
import ml_dtypes

import concourse.bass as bass
import concourse.mybir as mybir
import concourse.tile as tile
from concourse.bacc import Bacc

BF16 = mybir.dt.bfloat16
F32 = mybir.dt.float32
nbf16 = ml_dtypes.bfloat16

B = 2
S = 2048
D = 1024
H = 16
DH = 64
HPC = 4  # heads per core
CD = HPC * DH  # 256 per-core projected dims
NCORES = 8
SCALE = 8.0  # sqrt(DH)

KC = D // 128  # 8 contraction chunks for projections
TB = 512  # projection token block
NTB = S // TB  # 4
TC = S // 128  # 16 s_k chunks
SUP = 512  # q-superblock per attention instance
NSUP = S // SUP  # 4


def build_module(debug=False):
    nc = Bacc(None)

    xqT = nc.dram_tensor("xqT", [D, S], BF16, kind="ExternalInput")
    xkT = nc.dram_tensor("xkT", [D, S], BF16, kind="ExternalInput")
    xvT = nc.dram_tensor("xvT", [D, S], BF16, kind="ExternalInput")
    wqT = nc.dram_tensor("wqT", [128, KC * CD], BF16, kind="ExternalInput")
    wkT = nc.dram_tensor("wkT", [128, KC * CD], BF16, kind="ExternalInput")
    wvT = nc.dram_tensor("wvT", [128, KC * CD], BF16, kind="ExternalInput")
    woT = nc.dram_tensor("woT", [128, (CD // 128) * D], BF16, kind="ExternalInput")
    bqc = nc.dram_tensor("bqc", [128, 2], F32, kind="ExternalInput")
    bkc = nc.dram_tensor("bkc", [128, 2], F32, kind="ExternalInput")
    expbT = nc.dram_tensor("expbT", [S, S], BF16, kind="ExternalInput")
    poutT = nc.dram_tensor("poutT", [D, S], BF16, kind="ExternalOutput")

    with tile.TileContext(nc) as tc:
        with (
            tc.tile_pool(name="statics", bufs=1) as statics,
            tc.tile_pool(name="xk", bufs=16) as xk_pool,
            tc.tile_pool(name="xq", bufs=16) as xq_pool,
            tc.tile_pool(name="expb", bufs=2) as expb_pool,
            tc.tile_pool(name="e", bufs=3) as e_pool,
            tc.tile_pool(name="a", bufs=3) as a_pool,
            tc.tile_pool(name="o2", bufs=2) as o2_pool,
            tc.tile_pool(name="rb", bufs=2) as rb_pool,
            tc.tile_pool(name="spr", bufs=2) as spr_pool,
            tc.tile_pool(name="segt", bufs=2) as seg_pool,
            tc.tile_pool(name="oev", bufs=3) as oev_pool,
            tc.tile_pool(name="psc", bufs=2, space="PSUM") as psc,
            tc.tile_pool(name="pav", bufs=2, space="PSUM") as pav,
            tc.tile_pool(name="pfill", bufs=2, space="PSUM") as pfill,
            tc.tile_pool(name="dsc", bufs=4, space="DRAM") as dram_pool,
        ):
            # ---- static tiles ----
            wq_sb = statics.tile([128, KC, CD], BF16, name="wq_sb")
            wk_sb = statics.tile([128, KC, CD], BF16, name="wk_sb")
            wv_sb = statics.tile([128, KC, CD], BF16, name="wv_sb")
            wo_sb = statics.tile([128, CD // 128, D], BF16, name="wo_sb")
            bq_sb = statics.tile([128, 2], F32, name="bq_sb")
            bk_sb = statics.tile([128, 2], F32, name="bk_sb")
            qT = [statics.tile([128, S], BF16, name=f"qT{m}") for m in range(2)]
            kpair = [statics.tile([128, S], BF16, name=f"kpair{m}") for m in range(2)]
            vv = statics.tile([128, HPC, TC, DH + 1], BF16, name="vv")
            cc = [
                [
                    statics.tile([128, SUP], BF16, name=f"cc{m}_{s}")
                    for s in range(NSUP)
                ]
                for m in range(2)
            ]
            xv_sb = [statics.tile([128, S], BF16, name=f"xv{k}") for k in range(KC)]
            # junk tiles for PE warmup / ACT table preload
            wjl = statics.tile([128, 128], BF16, name="wjl")
            wjr = statics.tile([128, TB], BF16, name="wjr")
            ejunk = statics.tile([128, 64], BF16, name="ejunk")

            # x tiles: [128,512] quarters for the start-critical tb0, then
            # [128,1024] halves (256KB DMAs sustain higher aggregate BW)
            xkt2 = {}  # (tb, kc) -> AP
            xqt2 = {}

            def load_x_quarter(dst, xdram, pool, tb, nm):
                for kc in range(KC):
                    t = pool.tile([128, TB], BF16, name=nm, bufs=16)
                    nc.sync.dma_start(
                        t, xdram[kc * 128 : (kc + 1) * 128, tb * TB : (tb + 1) * TB]
                    )
                    dst[(tb, kc)] = t

            def load_x_half(dst, xdram, pool, h, nm):
                for kc in range(KC):
                    t = pool.tile([128, 2 * TB], BF16, name=nm, bufs=8)
                    nc.sync.dma_start(
                        t, xdram[kc * 128 : (kc + 1) * 128, h * 1024 : (h + 1) * 1024]
                    )
                    dst[(2 * h, kc)] = t[:, 0:TB]
                    dst[(2 * h + 1, kc)] = t[:, TB : 2 * TB]

            expb_tiles = {}
            expb_srcs = [
                expbT[:, sup * SUP : (sup + 1) * SUP].rearrange(
                    "(c p) q -> p c q", p=128
                )
                for sup in range(NSUP)
            ]

            def alloc_expb(sup):
                expb_tiles[sup] = expb_pool.tile([128, TC, SUP], BF16, name="eb")

            def load_expb(sup, c0, c1):  # chunks [c0, c1)
                nc.sync.dma_start(
                    expb_tiles[sup][:, c0:c1, :], expb_srcs[sup][:, c0:c1, :]
                )

            def load_xv_col(q0, q1):  # [128, (q1-q0)*512] col-block per kc
                for kc in range(KC):
                    nc.sync.dma_start(
                        xv_sb[kc][:, q0 * TB : q1 * TB],
                        xvT[kc * 128 : (kc + 1) * 128, q0 * TB : q1 * TB],
                    )

            # ---- front DMA stream (Sync is in-order; position ~= issue time)
            nc.sync.dma_start(wk_sb, wkT[:, :].rearrange("p (kc m) -> p kc m", kc=KC))
            nc.sync.dma_start(bk_sb, bkc[:, :])
            nc.sync.dma_start(bq_sb, bqc[:, :])
            load_x_quarter(xkt2, xkT, xk_pool, 0, "xkq")  # k-proj tb0
            alloc_expb(0)
            load_expb(0, 0, 2)
            nc.sync.dma_start(wq_sb, wqT[:, :].rearrange("p (kc m) -> p kc m", kc=KC))
            load_x_quarter(xqt2, xqT, xq_pool, 0, "xqq")  # q-proj tb0 (sup0)
            load_x_quarter(xkt2, xkT, xk_pool, 1, "xkq")  # k-proj tb1
            nc.sync.dma_start(wv_sb, wvT[:, :].rearrange("p (kc m) -> p kc m", kc=KC))
            load_xv_col(0, 2)  # v-proj tk0-7
            load_expb(0, 2, 4)
            load_expb(0, 4, 8)
            load_x_half(xkt2, xkT, xk_pool, 1, "xkh")  # k-proj tb2/tb3
            load_expb(0, 8, 12)
            load_xv_col(2, 4)  # v-proj tk8-15
            load_expb(0, 12, 16)
            nc.sync.dma_start(
                wo_sb, woT[:, :].rearrange("p (kc m) -> p kc m", kc=CD // 128)
            )
            alloc_expb(1)
            for c in range(0, TC, 4):
                load_expb(1, c, c + 4)
            # q-proj tb1/tb2 in one [128,1024] block, tb3 separate
            for kc in range(KC):
                t = xq_pool.tile([128, 2 * TB], BF16, name="xqm", bufs=8)
                nc.sync.dma_start(
                    t, xqT[kc * 128 : (kc + 1) * 128, TB : 3 * TB]
                )
                xqt2[(1, kc)] = t[:, 0:TB]
                xqt2[(2, kc)] = t[:, TB : 2 * TB]
            load_x_quarter(xqt2, xqT, xq_pool, 3, "xqq")  # q-proj tb3

            # ---- PE warmup (HAM) + ACT table preload ----
            nc.gpsimd.memset(wjl[:, :], 0.0)
            nc.gpsimd.memset(wjr[:, :], 0.0)
            nc.gpsimd.memset(vv[:, :, :, DH : DH + 1], 1.0)
            for i in range(12):
                ps = pfill.tile([128, TB], F32, name="ps_junk", tag="pfill", bufs=1)
                nc.tensor.matmul(ps, lhsT=wjl, rhs=wjr, start=True, stop=True)
            nc.scalar.activation(
                ejunk, wjr[:, 0:64], func=mybir.ActivationFunctionType.Exp
            )

            # ---- projection helpers ----
            kq_open = {}

            def kq_part(xt, w_sb, b_sb, dstt, tb, mt, part):
                # half-chain quantum: 4 of 8 accumulating MMs; the open PSUM
                # tile lives in its own single-buf tag so other fill quanta
                # cannot recycle it mid-accumulation
                key = (id(dstt), tb, mt)
                if part == 0:
                    kq_open[key] = pfill.tile(
                        [128, TB], F32, name="ps_kq", tag="pkq", bufs=1
                    )
                ps = kq_open[key]
                for kc in range(part * 4, part * 4 + 4):
                    nc.tensor.matmul(
                        ps,
                        lhsT=w_sb[:, kc, mt * 128 : (mt + 1) * 128],
                        rhs=xt[(tb, kc)],
                        start=(kc == 0),
                        stop=(kc == KC - 1),
                    )
                if part == 1:
                    nc.vector.tensor_scalar_add(
                        dstt[mt][:, tb * TB : (tb + 1) * TB],
                        ps,
                        scalar1=b_sb[:, mt : mt + 1],
                    )
                    del kq_open[key]

            def kq_chain(xt, w_sb, b_sb, dstt, tb, mt):
                kq_part(xt, w_sb, b_sb, dstt, tb, mt, 0)
                kq_part(xt, w_sb, b_sb, dstt, tb, mt, 1)

            def vproj_tile(tk):
                ps = pfill.tile([128, CD], F32, name="ps_v", tag="pfill", bufs=1)
                for kc in range(KC):
                    nc.tensor.matmul(
                        ps,
                        lhsT=xv_sb[kc][:, tk * 128 : (tk + 1) * 128],
                        rhs=wv_sb[:, kc, :],
                        start=(kc == 0),
                        stop=(kc == KC - 1),
                    )
                dst = vv[:, :, tk, 0:DH]
                src = ps.rearrange("p (h d) -> p h d", h=HPC)
                # ScalarE only while it is idle (prefix); DVE once exp stream runs
                if tk < 8 and tk % 2 == 0:
                    nc.scalar.copy(dst, src)
                else:
                    nc.vector.tensor_copy(dst, src)

            op_ctr = [0]

            def outproj_tile(mo, sup, alt=False):
                # alt: alternate fill tags (pkq is free once projections end)
                # so consecutive tiles double-buffer across the two banks
                if alt and op_ctr[0] % 2 == 1:
                    ps = pfill.tile([128, SUP], F32, name="ps_o2", tag="pkq", bufs=1)
                else:
                    ps = pfill.tile([128, SUP], F32, name="ps_o", tag="pfill", bufs=1)
                op_ctr[0] += 1
                for kc in range(CD // 128):
                    nc.tensor.matmul(
                        ps,
                        lhsT=wo_sb[:, kc, mo * 128 : (mo + 1) * 128],
                        rhs=cc[kc][sup][:, :],
                        start=(kc == 0),
                        stop=(kc == CD // 128 - 1),
                    )
                ot = oev_pool.tile([128, SUP], BF16, name="ot")
                nc.vector.tensor_copy(ot, ps)
                nc.sync.dma_start(
                    poutT[mo * 128 : (mo + 1) * 128, sup * SUP : (sup + 1) * SUP], ot
                )

            # ---- prefix projections (in column-arrival order) ----
            for mt in range(2):
                kq_chain(xkt2, wk_sb, bk_sb, kpair, 0, mt)
            for mt in range(2):
                kq_chain(xqt2, wq_sb, bq_sb, qT, 0, mt)
            for mt in range(2):
                kq_chain(xkt2, wk_sb, bk_sb, kpair, 1, mt)
            for tk in range(8):
                vproj_tile(tk)

            # ---- filler queue: drained 1 quantum per attention chunk ----
            fillers = []

            def kq2(xt, w, b, dstt, tb, mt):
                return [
                    lambda: kq_part(xt, w, b, dstt, tb, mt, 0),
                    lambda: kq_part(xt, w, b, dstt, tb, mt, 1),
                ]

            k2m0 = kq2(xkt2, wk_sb, bk_sb, kpair, 2, 0)
            k2m1 = kq2(xkt2, wk_sb, bk_sb, kpair, 2, 1)
            k3m0 = kq2(xkt2, wk_sb, bk_sb, kpair, 3, 0)
            k3m1 = kq2(xkt2, wk_sb, bk_sb, kpair, 3, 1)
            q1m0 = kq2(xqt2, wq_sb, bq_sb, qT, 1, 0)
            q1m1 = kq2(xqt2, wq_sb, bq_sb, qT, 1, 1)
            fillers += [k2m0[0], lambda: vproj_tile(8), k2m0[1],
                        lambda: vproj_tile(9), k3m0[0], lambda: vproj_tile(10),
                        k3m0[1], lambda: vproj_tile(11), lambda: vproj_tile(12),
                        lambda: vproj_tile(13), lambda: vproj_tile(14),
                        lambda: vproj_tile(15), k2m1[0], k2m1[1],
                        k3m1[0], k3m1[1], q1m0[0], q1m0[1], q1m1[0], q1m1[1]]

            fill_pos = [0]

            def pump_filler():
                if fill_pos[0] < len(fillers):
                    fillers[fill_pos[0]]()
                    fill_pos[0] += 1

            # ---- epilogue builder (A/B combined: one DMA chain per pair)
            def make_epi(sup, mt, out2A, out2B):
                qsl = slice(sup * SUP, (sup + 1) * SUP)
                st = {}
                stages = []

                def ev(which, out2):
                    if "o2" not in st:
                        st["o2"] = o2_pool.tile(
                            [DH + 1, 2 * SUP], F32, name="o2", tag="o2", bufs=1
                        )
                    half = 0 if which == "A" else 1
                    nc.vector.tensor_copy(
                        st["o2"][:, half * SUP : (half + 1) * SUP], out2
                    )

                stages.append(lambda: ev("A", out2A))
                stages.append(lambda: ev("B", out2B))

                def dspill():
                    dr = dram_pool.tile([1, 2 * SUP], F32, name="dr", tag="dr")
                    nc.sync.dma_start(dr, st["o2"][DH : DH + 1, :])
                    st["dr"] = dr

                stages.append(dspill)

                def dspread():
                    sp = spr_pool.tile([128, 2 * SUP // 128], F32, name="sp", tag="sp")
                    nc.sync.dma_start(
                        sp, st["dr"][:, :].rearrange("a (p f) -> (a p) f", p=128)
                    )
                    st["sp"] = sp

                stages.append(dspread)

                def recip():
                    sp = st["sp"]
                    nc.vector.reciprocal(sp, sp)
                    dr2 = dram_pool.tile([1, 2 * SUP], F32, name="dr2", tag="dr2")
                    nc.sync.dma_start(
                        dr2[:, :].rearrange("a (p f) -> (a p) f", p=128), sp
                    )
                    st["dr2"] = dr2

                stages.append(recip)

                def dbcast():
                    rb = rb_pool.tile([64, 2 * SUP], F32, name="rb", tag="rb", bufs=1)
                    nc.sync.dma_start(rb, st["dr2"][:, :].partition_broadcast(64))
                    st["rb"] = rb

                stages.append(dbcast)

                def finA():  # even head -> cc partitions 0-63 directly
                    seg = cc[mt][sup][0:64, :]
                    nc.vector.tensor_mul(
                        seg, st["o2"][0:DH, 0:SUP], st["rb"][:, 0:SUP]
                    )

                stages.append(finA)

                def finB():  # odd head -> partitions 64-127 via DMA move
                    segt = seg_pool.tile([64, SUP], BF16, name="segt")
                    nc.vector.tensor_mul(
                        segt, st["o2"][0:DH, SUP : 2 * SUP], st["rb"][:, SUP : 2 * SUP]
                    )
                    nc.sync.dma_start(cc[mt][sup][64:128, :], segt)

                stages.append(finB)
                return stages

            EPI_AT = {0: 0, 1: 1, 2: 2, 3: 3, 5: 4, 7: 5, 9: 6, 11: 7}

            # ---- attention instances ----
            pend = None
            carry = [{}]
            insts = [(sup, mt) for sup in range(NSUP) for mt in range(2)]
            for inst, (sup, mt) in enumerate(insts):
                qsl = slice(sup * SUP, (sup + 1) * SUP)
                hA, hB = 2 * mt, 2 * mt + 1

                # stream later expb superblocks while attention runs
                if inst == 2:
                    alloc_expb(2)
                    for c in range(0, TC, 4):
                        load_expb(2, c, c + 4)
                if inst == 2:
                    for mtq in range(2):
                        fillers.extend(kq2(xqt2, wq_sb, bq_sb, qT, 2, mtq))
                if inst == 3:
                    for mtq in range(2):
                        fillers.extend(kq2(xqt2, wq_sb, bq_sb, qT, 3, mtq))
                if inst == 4:
                    alloc_expb(3)
                    for c in range(0, TC, 4):
                        load_expb(3, c, c + 4)
                # output projection for finished superblocks (appended one
                # instance after the epilogue that completes cc, so the PE
                # queue never blocks on an epilogue still in flight)
                if inst in (4, 6):
                    s_done = {4: 0, 6: 1}[inst]
                    for mo in range(D // 128):
                        fillers.append(lambda mo=mo, s=s_done: outproj_tile(mo, s))

                def emit_sc(sup_, mt_, ck, t):
                    ksl = slice(ck * 128, (ck + 1) * 128)
                    qsl_ = slice(sup_ * SUP, (sup_ + 1) * SUP)
                    nc.tensor.matmul(
                        t[:, 0:SUP],
                        lhsT=kpair[mt_][0:64, ksl],
                        rhs=qT[mt_][0:64, qsl_],
                        start=True,
                        stop=True,
                    )
                    nc.tensor.matmul(
                        t[:, SUP : 2 * SUP],
                        lhsT=kpair[mt_][64:128, ksl],
                        rhs=qT[mt_][64:128, qsl_],
                        start=True,
                        stop=True,
                    )

                if inst == 0:
                    scs = {}
                    for ck in (0, 1):
                        scs[ck] = psc.tile([128, 2 * SUP], F32, name="sc", tag="psc")
                        emit_sc(sup, mt, ck, scs[ck])
                else:
                    scs = carry[0]
                    carry[0] = {}
                out2A = pav.tile([DH + 1, SUP], F32, name="o2A", tag="pav")
                out2B = pav.tile([DH + 1, SUP], F32, name="o2B", tag="pav")

                for ck in range(TC):
                    sc = scs.pop(ck)
                    e = e_pool.tile([128, 2 * SUP], BF16, name="e")
                    nc.scalar.activation(
                        e, sc, func=mybir.ActivationFunctionType.Exp
                    )
                    if pend is not None and ck in EPI_AT:
                        pend[EPI_AT[ck]]()
                    # one fused mul: expb chunk repeats across both heads
                    ebr = expb_tiles[sup][:, ck, None, :].broadcast_to([128, 2, SUP])
                    a = a_pool.tile([128, 2 * SUP], BF16, name="a")
                    nc.vector.tensor_mul(
                        a.rearrange("p (o f) -> p o f", o=2),
                        e.rearrange("p (o f) -> p o f", o=2),
                        ebr,
                    )
                    if ck + 2 < TC:
                        scs[ck + 2] = psc.tile(
                            [128, 2 * SUP], F32, name="sc", tag="psc"
                        )
                        emit_sc(sup, mt, ck + 2, scs[ck + 2])
                    elif inst + 1 < len(insts):
                        # prefetch next instance's first scores so the exp
                        # stream never waits on the instance boundary
                        nsup, nmt = insts[inst + 1]
                        nk = ck + 2 - TC
                        t = psc.tile([128, 2 * SUP], F32, name="sc", tag="psc")
                        emit_sc(nsup, nmt, nk, t)
                        carry[0][nk] = t
                    nc.tensor.matmul(
                        out2A,
                        lhsT=vv[:, hA, ck, :],
                        rhs=a[:, 0:SUP],
                        start=(ck == 0),
                        stop=(ck == TC - 1),
                    )
                    nc.tensor.matmul(
                        out2B,
                        lhsT=vv[:, hB, ck, :],
                        rhs=a[:, SUP : 2 * SUP],
                        start=(ck == 0),
                        stop=(ck == TC - 1),
                    )
                    if inst > 0 or ck >= 2:
                        pump_filler()

                pend = make_epi(sup, mt, out2A, out2B)

            # ---- tail: last epilogue chain (DVE/DMA) runs while PE does
            # sup2's output projection; sup3 follows once cc is complete
            while fill_pos[0] < len(fillers):
                pump_filler()
            # sup2 outproj first: emitted before the s3 epilogue's cc writes
            # so it carries no dependency on the (serial) DMA chain and keeps
            # PE busy underneath it
            for stage in pend[:2]:
                stage()
            for mo in range(D // 128):
                outproj_tile(mo, NSUP - 2, alt=True)
            for stage in pend[2:]:
                stage()
            pend = None
            for mo in range(D // 128):
                outproj_tile(mo, NSUP - 1, alt=True)

    nc.finalize()
    return nc


def make_in_maps(query, key, value, mask, chemical_bias, Wq, bq, Wk, bk, Wv, bv, Wo):
    """Host-side preprocessing: per-core input dicts (8 cores)."""
    f32 = np.float32

    def c(a, dt):
        return np.ascontiguousarray(a, dtype=dt)

    per_batch = []
    for b in range(B):
        xq = c(query[b].T, nbf16)
        xk = c(key[b].T, nbf16)
        xv = c(value[b].T, nbf16)
        bm = np.where(mask[b, 0] == 0, f32(0.0), np.exp(chemical_bias[b], dtype=f32))
        expbT_ = c(bm.T, nbf16)
        per_batch.append((xq, xk, xv, expbT_))

    def warr(wt, kc):
        # [kc*128, M] -> [128, kc*M]: per-partition-contiguous device layout
        m = wt.shape[1]
        return np.ascontiguousarray(
            wt.reshape(kc, 128, m).transpose(1, 0, 2).reshape(128, kc * m), nbf16
        )

    per_group = []
    for g in range(4):
        hsl = slice(g * CD, (g + 1) * CD)
        wqT_ = warr(np.asarray((Wq[hsl] / SCALE).T, np.float32), KC)
        wkT_ = warr(np.asarray(Wk[hsl].T, np.float32), KC)
        wvT_ = warr(np.asarray(Wv[hsl].T, np.float32), KC)
        woT_ = warr(np.asarray(Wo[:, hsl].T, np.float32), CD // 128)
        bqc_ = c((bq[hsl] / SCALE).reshape(2, 128).T, f32)
        bkc_ = c(bk[hsl].reshape(2, 128).T, f32)
        per_group.append((wqT_, wkT_, wvT_, woT_, bqc_, bkc_))

    in_maps = []
    for core in range(NCORES):
        b, g = divmod(core, 4)
        xq, xk, xv, expbT_ = per_batch[b]
        wqT_, wkT_, wvT_, woT_, bqc_, bkc_ = per_group[g]
        in_maps.append(
            {
                "xqT": xq,
                "xkT": xk,
                "xvT": xv,
                "wqT": wqT_,
                "wkT": wkT_,
                "wvT": wvT_,
                "woT": woT_,
                "bqc": bqc_,
                "bkc": bkc_,
                "expbT": expbT_,
            }
        )
    return in_maps


def combine_outputs(results, bo_eff):
    """Sum per-group transposed partials into the full [B, S, D] output."""
    out = np.empty((B, S, D), np.float32)
    for b in range(B):
        acc = results[4 * b]["poutT"].T.astype(np.float32)
        for g in range(1, 4):
            acc = acc + results[4 * b + g]["poutT"].T.astype(np.float32)
        out[b] = acc + bo_eff
    return out


_NC_CACHE = {}


def _get_module(debug=False):
    key = debug
    if key not in _NC_CACHE:
        _NC_CACHE[key] = build_module(debug=debug)
    return _NC_CACHE[key]


def run_spmd(in_maps, debug=False, **kwargs):
    from concourse.bass_utils import run_bass_kernel_spmd

    nc = _get_module(debug)
    return run_bass_kernel_spmd(nc, in_maps, core_ids=list(range(NCORES)), **kwargs)


def kernel(query, key, value, mask, chemical_bias, Wq, bq, Wk, bk, Wv, bv, Wo, bo):
    in_maps = make_in_maps(
        query, key, value, mask, chemical_bias, Wq, bq, Wk, bk, Wv, bv, Wo
    )
    res = run_spmd(in_maps)
    # v-bias folded out of the device epilogue: out += Wo @ bv per column
    bo_eff = (
        bo.astype(np.float32) + Wo.astype(np.float32) @ bv.astype(np.float32)
    )
    return combine_outputs(res.results, bo_eff)
